# revision 14
# baseline (speedup 1.0000x reference)
"""Trainium2 Bass kernel for the Performer-style random-feature map:

    out[n, s] = exp(-||x_n||^2 / 2) * S^{-1/2} * exp((x @ W.T)[n, s] + b[s])
              = exp((x @ W.T)[n, s] - 0.5*||x_n||^2 - 0.5*ln(S)) * exp(b[s])

Sharding: data-parallel over the N (row) axis across 8 NeuronCores; W and b
replicated.  Each core computes a [2048, 2048] output block.  Pure SPMD, no
collectives.

Per-core structure (sizes hardcoded for N=16384, D=1024, S=2048):
  - x^T and W^T live in SBUF as fp8e4 (W pre-scaled by 16 on the host to
    stay out of the fp8 subnormal range); matmuls run in
    MatmulPerfMode.DoubleRow (two 128-deep k-subtiles per instruction,
    ~1.7x bf16 throughput measured).  Operands stream in per-k-subtile
    chunks on the sync (x) and scalar (W) DMA rings so the PE starts
    ~2us in; subtile dependency tracking lets each matmul wait only on
    the chunks it reads.  SBUF layouts keep each matmul operand slice
    contiguous ([P, grp, KT, width]).
  - the per-row bias -0.5*||x_n||^2 - 0.5*ln(S) rides in as a [NCc] f32
    vector (host-packed [128, NB]), so no second copy of x is loaded.
  - per [128, 1024] PSUM pair-bank tile: 8 DoubleRow matmuls -> one ACT
    exp(psum/16 + bias_n) -> bf16 tmp; one 2048-wide DVE multiply by
    exp(b) per row block -> bf16 DMA out on alternating rings (host
    upcasts to f32).
"""

import sys
from contextlib import ExitStack

if "/opt/trn_rl_repo" not in sys.path:
    sys.path.insert(0, "/opt/trn_rl_repo")

import numpy as np

import concourse.bacc as bacc
import concourse.bass as bass
import concourse.tile as tile
from concourse import mybir

P = 128          # SBUF partitions
N_FULL = 16384   # total rows
D_FULL = 1024    # contraction dim
S_FULL = 2048    # output features
N_CORES = 8
NC_FULL = N_FULL // N_CORES  # rows per core

F32 = mybir.dt.float32
BF16 = mybir.dt.bfloat16
FP8 = mybir.dt.float8e4

W_SCALE = 16.0   # host multiplies W by this before fp8 cast


def build_nc(NCc=NC_FULL, D=D_FULL, S=S_FULL, psum_w=1024, warmup=16,
             mul_wide=True):
    """Build the single-core Bass program (same program runs SPMD on 8 cores)."""
    nc = bacc.Bacc("TRN2", target_bir_lowering=False, debug=False)

    xT = nc.dram_tensor("xT", [D, NCc], FP8, kind="ExternalInput").ap()
    w = nc.dram_tensor("w", [D, S], FP8, kind="ExternalInput").ap()
    bv = nc.dram_tensor("bias", [S], F32, kind="ExternalInput").ap()
    # host-packed [-0.5*||x_n||^2 - 0.5*ln(S)] as [P, NB]
    rb = nc.dram_tensor("rowbias", [P, NCc // P], F32,
                        kind="ExternalInput").ap()
    out = nc.dram_tensor("out", [NCc, S], BF16, kind="ExternalOutput").ap()

    KT = D // P            # k subtiles (contraction)
    KP = KT // 2           # k pairs (DoubleRow consumes 2 subtiles)
    NB = NCc // P          # 128-row output blocks
    NS = 512               # matmul moving free width (f32 psum half-bank pair)
    SW = psum_w            # psum tile width (2 banks)
    SH = S // SW           # psum tiles per row block
    CH = SW // NS          # matmul column groups per psum tile
    DR = mybir.MatmulPerfMode.DoubleRow

    with tile.TileContext(nc) as tc, ExitStack() as ctx:
        singles = ctx.enter_context(tc.tile_pool(name="singles", bufs=1))
        # w layout keeps the matmul rhs slice [2, 512] contiguous (the moving
        # feed needs adjacent k-pairs to double-pump); x keeps 2KB DMA runs:
        #   w_sb[p, c, k, j] = W[k*128+p, c*512+j]
        #   x_sb[p, k, n]    = x[n, k*128+p]
        w_sb = singles.tile([P, S // NS, KT, NS], FP8)
        x_sb = singles.tile([P, KT, NCc], FP8)
        b_bc = singles.tile([P, S], F32)
        eb = singles.tile([P, S], BF16)
        rb_sb = singles.tile([P, NB], F32)

        # --- DMA issue ---
        # scalar ring: b broadcast, row-bias, W k-chunks (+ half the outs);
        # sync ring: x k-chunks (+ half the outs).
        bv_bcast = bass.AP(tensor=bv.tensor, offset=bv.offset,
                           ap=[[0, P]] + list(bv.ap))
        nc.scalar.dma_start(b_bc, bv_bcast)
        nc.scalar.dma_start(rb_sb, rb)
        # Demand-ordered operand streaming in ~64-128KB chunks: per-DMA-engine
        # transfer rate is ~40GB/s with ~5 transfers in flight per ring, and
        # each dma_start costs ~0.6us of engine issue time.  Order of needs:
        # the first two row-blocks' x slices, all of W (needed in full by
        # every block), then the rest of x in all-k 256-column strips so
        # block j's operands always land well before the PE reaches it.
        XC = 2 * P                  # x columns on the critical path
        for kp in range(KP):
            eng = nc.sync if kp % 2 == 0 else nc.scalar
            eng.dma_start(
                x_sb[:, 2 * kp:2 * kp + 2, 0:XC],
                xT[2 * kp * P:(2 * kp + 2) * P, 0:XC].rearrange(
                    "(k p) n -> p k n", p=P))
        WH = 2                      # w column halves (psum-tile groups)
        cph = (S // NS) // WH
        for k in range(KT):
            for wh in range(WH):
                eng = nc.sync if (k + wh) % 2 == 0 else nc.scalar
                eng.dma_start(
                    w_sb[:, wh * cph:(wh + 1) * cph, k, :],
                    w[k * P:(k + 1) * P, wh * cph * NS:(wh + 1) * cph * NS]
                    .rearrange("p (c j) -> p c j", j=NS))
        for st in range(XC, NCc, XC):  # remaining x, strip by strip
            for kp in range(KP):
                eng = nc.sync if kp % 2 == 0 else nc.scalar
                eng.dma_start(
                    x_sb[:, 2 * kp:2 * kp + 2, st:st + XC],
                    xT[2 * kp * P:(2 * kp + 2) * P, st:st + XC].rearrange(
                        "(k p) n -> p k n", p=P))

        nc.scalar.activation(eb, b_bc, func=mybir.ActivationFunctionType.Exp)

        psum_pool = ctx.enter_context(
            tc.tile_pool(name="psum", bufs=8 * 512 // SW, space="PSUM"))
        tmp_pool = ctx.enter_context(tc.tile_pool(name="tmp", bufs=3))
        out_pool = ctx.enter_context(tc.tile_pool(name="osb", bufs=4))

        if warmup:
            # keep the PE busy (and the pstate ramping) while the first
            # operand chunks stream in; results are discarded
            dummy_x = singles.tile([P, 2, P], FP8)
            dummy_w = singles.tile([P, 2, NS], FP8)
            nc.vector.memset(dummy_x, 0.0)
            nc.vector.memset(dummy_w, 0.0)
            for i in range(warmup):
                wps = psum_pool.tile([P, SW], F32, tag="ps", name=f"warm{i}")
                nc.tensor.matmul(wps[:, 0:NS], lhsT=dummy_x, rhs=dummy_w,
                                 start=True, stop=True, perf_mode=DR)

        for nb in range(NB):
            tmp = tmp_pool.tile([P, S], BF16)
            for h in range(SH):
                ps = psum_pool.tile([P, SW], F32, tag="ps", name=f"ps{nb}_{h}")
                for kp in range(KP):
                    for c in range(CH):
                        nc.tensor.matmul(
                            ps[:, c * NS:(c + 1) * NS],
                            lhsT=x_sb[:, 2 * kp:2 * kp + 2,
                                      nb * P:(nb + 1) * P],
                            rhs=w_sb[:, h * CH + c, 2 * kp:2 * kp + 2, :],
                            start=(kp == 0),
                            stop=(kp == KP - 1),
                            perf_mode=DR,
                        )
                nc.scalar.activation(
                    tmp[:, h * SW:(h + 1) * SW], ps,
                    func=mybir.ActivationFunctionType.Exp,
                    bias=rb_sb[:, nb:nb + 1],
                    scale=1.0 / W_SCALE,
                )
            o_sb = out_pool.tile([P, S], BF16)
            last = nb == NB - 1
            if mul_wide and not last:
                nc.vector.tensor_mul(o_sb, tmp, eb)
                out_eng = nc.sync if nb % 2 == 0 else nc.scalar
                out_eng.dma_start(out[nb * P:(nb + 1) * P, :], o_sb)
            else:
                # fine-grained drain (shorter tail on the final block):
                # small DMAs spread over both rings finish sooner
                for h in range(SH):
                    sl = slice(h * SW, (h + 1) * SW)
                    nc.vector.tensor_mul(o_sb[:, sl], tmp[:, sl], eb[:, sl])
                    for piece in range(4):
                        psl = slice(h * SW + piece * SW // 4,
                                    h * SW + (piece + 1) * SW // 4)
                        out_eng = nc.sync if piece % 2 == 0 else nc.scalar
                        out_eng.dma_start(out[nb * P:(nb + 1) * P, psl],
                                          o_sb[:, psl])

    nc.compile()
    return nc


_NC_CACHE = {}


def _get_nc(**kwargs):
    key = tuple(sorted(kwargs.items()))
    if key not in _NC_CACHE:
        _NC_CACHE[key] = build_nc(**kwargs)
    return _NC_CACHE[key]


def make_in_maps(x, W, b):
    import ml_dtypes
    fp8 = ml_dtypes.float8_e4m3fn
    NB = NC_FULL // P
    wT = np.ascontiguousarray((W.T * W_SCALE).astype(fp8))
    b = np.ascontiguousarray(b.astype(np.float32))
    in_maps = []
    for i in range(N_CORES):
        xs = x[i * NC_FULL:(i + 1) * NC_FULL].astype(np.float32)
        rowbias = (-0.5 * (xs * xs).sum(axis=1)
                   - 0.5 * np.log(S_FULL)).astype(np.float32)
        in_maps.append({
            "xT": np.ascontiguousarray(xs.T.astype(fp8)),
            "w": wT,
            "bias": b,
            "rowbias": np.ascontiguousarray(rowbias.reshape(NB, P).T),
        })
    return in_maps


def run_hw(x, W, b, trace=False, **build_kwargs):
    """Run on 8 NeuronCores; returns (out [N, S] f32, BassKernelResults)."""
    from concourse.bass_utils import run_bass_kernel_spmd
    from concourse.bass_interp import get_hw_module

    nc = _get_nc(**build_kwargs)
    in_maps = make_in_maps(x, W, b)
    old_m = nc.m
    nc.m = get_hw_module(nc.m)
    try:
        res = run_bass_kernel_spmd(
            nc, in_maps, core_ids=list(range(N_CORES)), trace=trace)
    finally:
        nc.m = old_m
    out = np.concatenate(
        [res.results[i]["out"].astype(np.float32) for i in range(N_CORES)],
        axis=0)
    return out, res


def kernel(x, W, b):
    out, _ = run_hw(x, W, b, trace=False)
    return out


# revision 16
# speedup vs baseline: 1.2649x; 1.2649x over previous
"""Trainium2 Bass kernel for the Performer-style random-feature map:

    out[n, s] = exp(-||x_n||^2 / 2) * S^{-1/2} * exp((x @ W.T)[n, s] + b[s])
              = exp((x @ W.T)[n, s] - 0.5*||x_n||^2 - 0.5*ln(S)) * exp(b[s])

Sharding: data-parallel over the N (row) axis across 8 NeuronCores; W and b
replicated.  Each core computes a [2048, 2048] output block.  Pure SPMD, no
collectives.

Per-core structure (sizes hardcoded for N=16384, D=1024, S=2048):
  - x^T and W^T live in SBUF as fp8e4 (W pre-scaled by 16 on the host to
    stay out of the fp8 subnormal range); matmuls run in
    MatmulPerfMode.DoubleRow (two 128-deep k-subtiles per instruction,
    ~1.7x bf16 throughput measured).  Operands stream in per-k-subtile
    chunks on the sync (x) and scalar (W) DMA rings so the PE starts
    ~2us in; subtile dependency tracking lets each matmul wait only on
    the chunks it reads.  SBUF layouts keep each matmul operand slice
    contiguous ([P, grp, KT, width]).
  - the per-row bias -0.5*||x_n||^2 - 0.5*ln(S) rides in as a [NCc] f32
    vector (host-packed [128, NB]), so no second copy of x is loaded.
  - per [128, 1024] PSUM pair-bank tile: 8 DoubleRow matmuls -> one ACT
    exp(psum/16 + bias_n) -> bf16 tmp; one 2048-wide DVE multiply by
    exp(b) per row block -> bf16 DMA out on alternating rings (host
    upcasts to f32).
"""

import sys
from contextlib import ExitStack

if "/opt/trn_rl_repo" not in sys.path:
    sys.path.insert(0, "/opt/trn_rl_repo")

import numpy as np

import concourse.bacc as bacc
import concourse.bass as bass
import concourse.tile as tile
from concourse import mybir

P = 128          # SBUF partitions
N_FULL = 16384   # total rows
D_FULL = 1024    # contraction dim
S_FULL = 2048    # output features
N_CORES = 8
NC_FULL = N_FULL // N_CORES  # rows per core

F32 = mybir.dt.float32
BF16 = mybir.dt.bfloat16
FP8 = mybir.dt.float8e4

W_SCALE = 16.0   # host multiplies W by this before fp8 cast


def build_nc(NCc=NC_FULL, D=D_FULL, S=S_FULL, psum_w=1024, warmup=12,
             mul_wide=True):
    """Build the single-core Bass program (same program runs SPMD on 8 cores)."""
    nc = bacc.Bacc("TRN2", target_bir_lowering=False, debug=False)

    xT = nc.dram_tensor("xT", [D, NCc], FP8, kind="ExternalInput").ap()
    w = nc.dram_tensor("w", [D, S], FP8, kind="ExternalInput").ap()
    bv = nc.dram_tensor("bias", [S], F32, kind="ExternalInput").ap()
    # host-packed [-0.5*||x_n||^2 - 0.5*ln(S)] as [P, NB]
    rb = nc.dram_tensor("rowbias", [P, NCc // P], F32,
                        kind="ExternalInput").ap()
    out = nc.dram_tensor("out", [NCc, S], BF16, kind="ExternalOutput").ap()

    KT = D // P            # k subtiles (contraction)
    KP = KT // 2           # k pairs (DoubleRow consumes 2 subtiles)
    NB = NCc // P          # 128-row output blocks
    NS = 512               # matmul moving free width (f32 psum half-bank pair)
    SW = psum_w            # psum tile width (2 banks)
    SH = S // SW           # psum tiles per row block
    CH = SW // NS          # matmul column groups per psum tile
    DR = mybir.MatmulPerfMode.DoubleRow

    with tile.TileContext(nc) as tc, ExitStack() as ctx:
        singles = ctx.enter_context(tc.tile_pool(name="singles", bufs=1))
        # w layout keeps the matmul rhs slice [2, 512] contiguous (the moving
        # feed needs adjacent k-pairs to double-pump); x keeps 2KB DMA runs:
        #   w_sb[p, c, k, j] = W[k*128+p, c*512+j]
        #   x_sb[p, k, n]    = x[n, k*128+p]
        w_sb = singles.tile([P, S // NS, KT, NS], FP8)
        x_sb = singles.tile([P, KT, NCc], FP8)
        b_bc = singles.tile([P, S], F32)
        eb = singles.tile([P, S], BF16)
        rb_sb = singles.tile([P, NB], F32)

        # --- DMA issue ---
        # scalar ring: b broadcast, row-bias, W k-chunks (+ half the outs);
        # sync ring: x k-chunks (+ half the outs).
        bv_bcast = bass.AP(tensor=bv.tensor, offset=bv.offset,
                           ap=[[0, P]] + list(bv.ap))
        nc.scalar.dma_start(b_bc, bv_bcast)
        nc.scalar.dma_start(rb_sb, rb)
        # Demand-ordered operand streaming in ~64-128KB chunks: per-DMA-engine
        # transfer rate is ~40GB/s with ~5 transfers in flight per ring, and
        # each dma_start costs ~0.6us of engine issue time.  Order of needs:
        # the first two row-blocks' x slices, all of W (needed in full by
        # every block), then the rest of x in all-k 256-column strips so
        # block j's operands always land well before the PE reaches it.
        XC = 2 * P                  # x columns on the critical path
        nc.sync.dma_start(
            x_sb[:, 0:KT // 2, 0:XC],
            xT[0:KT // 2 * P, 0:XC].rearrange("(k p) n -> p k n", p=P))
        nc.scalar.dma_start(
            x_sb[:, KT // 2:KT, 0:XC],
            xT[KT // 2 * P:KT * P, 0:XC].rearrange("(k p) n -> p k n", p=P))
        WH = 2                      # w column halves (psum-tile groups)
        cph = (S // NS) // WH
        for k in range(KT):
            for wh in range(WH):
                eng = nc.sync if (k + wh) % 2 == 0 else nc.scalar
                eng.dma_start(
                    w_sb[:, wh * cph:(wh + 1) * cph, k, :],
                    w[k * P:(k + 1) * P, wh * cph * NS:(wh + 1) * cph * NS]
                    .rearrange("p (c j) -> p c j", j=NS))
        xm = (NCc - XC) // 2        # remaining x: two n-half sweeps, k-major
        for half in range(2):
            lo = XC + half * xm
            for k in range(KT):
                eng = nc.sync if (k + half) % 2 == 0 else nc.scalar
                eng.dma_start(
                    x_sb[:, k, lo:lo + xm], xT[k * P:(k + 1) * P, lo:lo + xm])

        nc.scalar.activation(eb, b_bc, func=mybir.ActivationFunctionType.Exp)

        psum_pool = ctx.enter_context(
            tc.tile_pool(name="psum", bufs=8 * 512 // SW, space="PSUM"))
        tmp_pool = ctx.enter_context(tc.tile_pool(name="tmp", bufs=3))
        out_pool = ctx.enter_context(tc.tile_pool(name="osb", bufs=4))

        if warmup:
            # keep the PE busy (and the pstate ramping) while the first
            # operand chunks stream in; results are discarded
            dummy_x = singles.tile([P, 2, P], FP8)
            dummy_w = singles.tile([P, 2, NS], FP8)
            nc.vector.memset(dummy_x, 0.0)
            nc.vector.memset(dummy_w, 0.0)
            for i in range(warmup):
                wps = psum_pool.tile([P, SW], F32, tag="ps", name=f"warm{i}")
                nc.tensor.matmul(wps[:, 0:NS], lhsT=dummy_x, rhs=dummy_w,
                                 start=True, stop=True, perf_mode=DR)

        for nb in range(NB):
            tmp = tmp_pool.tile([P, S], BF16)
            for h in range(SH):
                ps = psum_pool.tile([P, SW], F32, tag="ps", name=f"ps{nb}_{h}")
                for kp in range(KP):
                    for c in range(CH):
                        nc.tensor.matmul(
                            ps[:, c * NS:(c + 1) * NS],
                            lhsT=x_sb[:, 2 * kp:2 * kp + 2,
                                      nb * P:(nb + 1) * P],
                            rhs=w_sb[:, h * CH + c, 2 * kp:2 * kp + 2, :],
                            start=(kp == 0),
                            stop=(kp == KP - 1),
                            perf_mode=DR,
                        )
                nc.scalar.activation(
                    tmp[:, h * SW:(h + 1) * SW], ps,
                    func=mybir.ActivationFunctionType.Exp,
                    bias=rb_sb[:, nb:nb + 1],
                    scale=1.0 / W_SCALE,
                )
            o_sb = out_pool.tile([P, S], BF16)
            last = nb == NB - 1
            if mul_wide and not last:
                nc.vector.tensor_mul(o_sb, tmp, eb)
                out_eng = nc.sync if nb % 2 == 0 else nc.scalar
                out_eng.dma_start(out[nb * P:(nb + 1) * P, :], o_sb)
            else:
                # fine-grained drain (shorter tail on the final block):
                # small DMAs spread over both rings finish sooner
                for h in range(SH):
                    sl = slice(h * SW, (h + 1) * SW)
                    nc.vector.tensor_mul(o_sb[:, sl], tmp[:, sl], eb[:, sl])
                    for piece in range(4):
                        psl = slice(h * SW + piece * SW // 4,
                                    h * SW + (piece + 1) * SW // 4)
                        out_eng = nc.sync if piece % 2 == 0 else nc.scalar
                        out_eng.dma_start(out[nb * P:(nb + 1) * P, psl],
                                          o_sb[:, psl])

    nc.compile()
    return nc


_NC_CACHE = {}


def _get_nc(**kwargs):
    key = tuple(sorted(kwargs.items()))
    if key not in _NC_CACHE:
        _NC_CACHE[key] = build_nc(**kwargs)
    return _NC_CACHE[key]


def make_in_maps(x, W, b):
    import ml_dtypes
    fp8 = ml_dtypes.float8_e4m3fn
    NB = NC_FULL // P
    wT = np.ascontiguousarray((W.T * W_SCALE).astype(fp8))
    b = np.ascontiguousarray(b.astype(np.float32))
    in_maps = []
    for i in range(N_CORES):
        xs = x[i * NC_FULL:(i + 1) * NC_FULL].astype(np.float32)
        rowbias = (-0.5 * (xs * xs).sum(axis=1)
                   - 0.5 * np.log(S_FULL)).astype(np.float32)
        in_maps.append({
            "xT": np.ascontiguousarray(xs.T.astype(fp8)),
            "w": wT,
            "bias": b,
            "rowbias": np.ascontiguousarray(rowbias.reshape(NB, P).T),
        })
    return in_maps


def run_hw(x, W, b, trace=False, **build_kwargs):
    """Run on 8 NeuronCores; returns (out [N, S] f32, BassKernelResults)."""
    from concourse.bass_utils import run_bass_kernel_spmd
    from concourse.bass_interp import get_hw_module

    nc = _get_nc(**build_kwargs)
    in_maps = make_in_maps(x, W, b)
    old_m = nc.m
    nc.m = get_hw_module(nc.m)
    try:
        res = run_bass_kernel_spmd(
            nc, in_maps, core_ids=list(range(N_CORES)), trace=trace)
    finally:
        nc.m = old_m
    out = np.concatenate(
        [res.results[i]["out"].astype(np.float32) for i in range(N_CORES)],
        axis=0)
    return out, res


def kernel(x, W, b):
    out, _ = run_hw(x, W, b, trace=False)
    return out
